# revision 29
# baseline (speedup 1.0000x reference)
"""Distributed Trainium2 Bass kernel for a causal attention block + LayerNorm.

Reference computation (B=2, T=2048, C=1024, H=16 heads, Dh=64):
    q,k,v = x@Wq+bq, x@Wk+bk, x@Wv+bv          (per-head split)
    att   = softmax(causal(q k^T / sqrt(Dh)))
    o     = att @ v ; y = o@Wo + bo ; out = LayerNorm(y) * gamma + beta

Sharding (8 cores, one TRN2 chip):
    Tensor-parallel over heads: core i owns heads {2i, 2i+1} for BOTH
    batches (Megatron-style column shards of Wq/Wk/Wv).  After attention,
    two 8-core AllToAlls (one per local head, bf16 payload) redistribute the
    per-head outputs to token-sharding: core i ends with tokens
    [b = i//4, t in (i%4)*512 ...] with ALL 1024 features, applies the
    output projection (full Wo), bias and LayerNorm locally, and writes its
    (512, 1024) slice of the output.

Key scheduling facts this version is built around (measured on HW):
    - cores launch with ~80us of skew; every collective rendezvous waits for
      the straggler.  The wall time is ~(skew + per-core serial path), so the
      kernel minimizes the per-core path and keeps all collective-dependent
      work strictly AFTER independent work on every engine queue.
    - the Tile scheduler trusts its (fast) collective cost model and will
      happily slot post-collective ops between attention ops on the in-order
      engine queues; on HW they block on the real collective and stall the
      rest of attention.  tc.tile_wait_until() pushes the tail out.
    - softmax normalization is applied at the SOURCE core: denominators for
      all four q-blocks of a (head, batch) are gathered into one tile and
      inverted with a single exact DVE reciprocal (the HW op costs ~3.3us
      regardless of partition count), broadcast across partitions with a
      K=1 outer-product matmul on the PE, and multiplied in on DVE -- so
      after the AllToAlls only DMAs + the output projection remain.

Layout choices (all on-chip matmuls contract over the partition axis):
    - activations are feature-major: host passes x^T [C, B, T].
    - q^T,k^T,v^T [d, t] produced directly; v transposed on the PE into
      s-major v-hat [s, d] with an extra ones column per head so the P@V
      matmul also yields the softmax denominator for free.
    - scores are computed transposed: S^T[s, q] = k^T.T @ q^T; score chunks
      are packed in pairs into 2-bank PSUM tiles so each scalar-engine Exp
      call covers up to 1024 columns; causal masking via a triangular
      bf16 multiply on the 128-wide diagonal blocks only (on DVE).
"""

import numpy as np
import ml_dtypes

import concourse.bass as bass
import concourse.mybir as mybir
import concourse.tile as tile
from concourse import bacc
from concourse.bass_utils import run_bass_kernel_spmd
F32 = mybir.dt.float32
BF16 = mybir.dt.bfloat16
AF = mybir.ActivationFunctionType
OP = mybir.AluOpType

B, T, C, H, Dh = 2, 2048, 1024, 16, 64
NCORES = 8
HPC = 2               # heads per core
DPC = HPC * Dh        # 128 feature columns per core
TS = 512              # output token-slice length per core
NQB = T // 512        # 4 q blocks
NST = T // 128        # 16 s tiles
NCT = C // 128        # 8 contraction tiles
EPS = 1e-5

DT_X = BF16
DT_W = BF16
DT_P = BF16
DT_A2A = BF16         # AllToAll payload dtype
NP_X = ml_dtypes.bfloat16
NP_W = ml_dtypes.bfloat16

_CACHE = {}


def _build():
    nc = bacc.Bacc("TRN2", target_bir_lowering=False, debug=False,
                   num_devices=NCORES)

    xT_h = nc.dram_tensor("xT", [128, NCT, B, T], DT_X, kind="ExternalInput")
    wq_h = nc.dram_tensor("wq", [128, NCT, DPC], DT_W, kind="ExternalInput")
    wk_h = nc.dram_tensor("wk", [128, NCT, DPC], DT_W, kind="ExternalInput")
    wv_h = nc.dram_tensor("wv", [128, NCT, DPC], DT_W, kind="ExternalInput")
    wo_h = nc.dram_tensor("wo", [128, NCT, C], DT_W, kind="ExternalInput")
    bqT_h = nc.dram_tensor("bqT", [DPC, 1], F32, kind="ExternalInput")
    bkT_h = nc.dram_tensor("bkT", [DPC, 1], F32, kind="ExternalInput")
    bvT_h = nc.dram_tensor("bvT", [DPC, 1], F32, kind="ExternalInput")
    bo_h = nc.dram_tensor("bo_row", [1, C], BF16, kind="ExternalInput")
    gam_h = nc.dram_tensor("gamb", [128, C], BF16, kind="ExternalInput")
    bet_h = nc.dram_tensor("betb", [128, C], BF16, kind="ExternalInput")
    out_h = nc.dram_tensor("out", [TS, C], BF16, kind="ExternalOutput")

    ones1_d = nc.inline_tensor(np.ones((1, 128), ml_dtypes.bfloat16), name="ones1_const")
    ident_d = nc.inline_tensor(
        np.eye(128, dtype=ml_dtypes.bfloat16), name="ident_const")
    tri_np = (np.tril(np.ones((128, 128), np.float32)).T).astype(ml_dtypes.bfloat16)
    tri_d = nc.inline_tensor(tri_np, name="tri_const")

    with tile.TileContext(nc) as tc:
        with (
            tc.tile_pool(name="const", bufs=1) as cp,
            tc.tile_pool(name="dram", bufs=1, space="DRAM") as dp,
            tc.tile_pool(name="act", bufs=1) as ap,
            tc.tile_pool(name="wop", bufs=1) as wop,
            tc.tile_pool(name="pp", bufs=5) as pp,
            tc.tile_pool(name="vtp", bufs=2) as vtp,
            tc.tile_pool(name="ohp", bufs=18) as ohp,
            tc.tile_pool(name="rp", bufs=4) as rp,
            tc.tile_pool(name="xw", bufs=1) as xw,
            tc.tile_pool(name="lnp", bufs=2) as lnp,
            tc.tile_pool(name="psM", bufs=2, space="PSUM") as psM,
            tc.tile_pool(name="psS2", bufs=2, space="PSUM") as psS2,
            tc.tile_pool(name="psOC", bufs=2, space="PSUM") as psOC,
        ):
            # ---- weights + first x chunk on the sync queue (needed first),
            # rest of x^T + output-side weights on the gpsimd queue ----
            wq = xw.tile([128, NCT, DPC], DT_W)
            wk = xw.tile([128, NCT, DPC], DT_W)
            wv = xw.tile([128, NCT, DPC], DT_W)
            xT = xw.tile([128, NCT, B, T], DT_X)
            nc.gpsimd.dma_start(wv[:], wv_h[:])
            bqT = cp.tile([DPC, 1], F32)
            nc.scalar.dma_start(bqT[:], bqT_h[:])
            bkT = cp.tile([DPC, 1], F32)
            nc.scalar.dma_start(bkT[:], bkT_h[:])
            bvT = cp.tile([DPC, 1], F32)
            nc.scalar.dma_start(bvT[:], bvT_h[:])
            ident = cp.tile([128, 128], BF16)
            nc.scalar.dma_start(ident[:], ident_d[:])
            ones1 = cp.tile([1, 128], BF16)
            nc.scalar.dma_start(ones1[:], ones1_d[:])
            tri = cp.tile([128, 128], BF16)
            nc.scalar.dma_start(tri[:], tri_d[:])
            # preload the Exp activation table before the first score chunk
            # needs it (saves the 1.3us load on the first chunk's chain)
            expw = cp.tile([1, 1], F32)
            nc.scalar.activation(expw[:], ones1[0:1, 0:1], AF.Exp)
            nc.sync.dma_start(wq[:], wq_h[:])
            nc.sync.dma_start(xT[:, 0, 0, 0:512], xT_h[:, 0, 0, 0:512])
            nc.sync.dma_start(wk[:], wk_h[:])
            for ct in range(1, NCT):
                nc.sync.dma_start(xT[:, ct, 0, 0:512], xT_h[:, ct, 0, 0:512])

            # tiny warm-up AllToAll: trigger it before the bulk DMA issues
            # clog the gpsimd queue, so the first-collective overhead is paid
            # as early as possible
            warm_in = dp.tile([NCORES, 8], F32, tag="wi")
            warm_out = dp.tile([NCORES, 8], F32, tag="wo_")
            warm_sb = cp.tile([NCORES, 8], F32)
            nc.gpsimd.memset(warm_sb[:], 0.0)
            nc.gpsimd.dma_start(warm_in[:], warm_sb[:])
            nc.gpsimd.collective_compute(
                "AllToAll", OP.bypass, replica_groups=[list(range(NCORES))],
                ins=[warm_in.opt()], outs=[warm_out.opt()])

            # split the b0-remainder stream across two DMA queues so the
            # projection loop never outruns the x data
            for ct in range(NCT):
                q = nc.gpsimd if ct % 2 == 0 else nc.scalar
                q.dma_start(xT[:, ct, 0, 512:T], xT_h[:, ct, 0, 512:T])
            for ct in range(NCT):
                nc.gpsimd.dma_start(xT[:, ct, 1, :], xT_h[:, ct, 1, :])

            # output-side weights, early (small; they only cost SBUF)
            wo = wop.tile([128, NCT, C], DT_W, tag="wo")
            nc.gpsimd.dma_start(wo[:], wo_h[:])
            bo = wop.tile([1, C], BF16, tag="bo")
            nc.gpsimd.dma_start(bo[:], bo_h[:])
            gam = wop.tile([128, C], BF16, tag="gam")
            nc.gpsimd.dma_start(gam[:], gam_h[:])
            bet = wop.tile([128, C], BF16, tag="bet")
            nc.gpsimd.dma_start(bet[:], bet_h[:])
            eps_t = wop.tile([128, 1], F32, tag="eps")
            nc.gpsimd.memset(eps_t[:], EPS)

            # ---- persistent activation tiles ----
            qT = ap.tile([DPC, B, T], DT_P)
            kT = ap.tile([DPC, B, T], DT_P)
            vhat = ap.tile([128, B, NST, HPC, 65], DT_P)
            oT = ap.tile([128, NCT, 512], DT_P)
            for b in range(B):
                nc.gpsimd.memset(vhat[:, b, :, :, 64:65], 1.0)

            a2a_in0 = dp.tile([NCORES, 64, 512], DT_A2A, tag="ai0")
            a2a_in1 = dp.tile([NCORES, 64, 512], DT_A2A, tag="ai1")
            a2a_out0 = dp.tile([NCORES, 64, 512], DT_A2A, tag="ao0")
            a2a_out1 = dp.tile([NCORES, 64, 512], DT_A2A, tag="ao1")
            a2a_in = [a2a_in0, a2a_in1]
            a2a_out = [a2a_out0, a2a_out1]

            def proj(b, qb):
                sl = slice(qb * 512, (qb + 1) * 512)
                for w_sb, bias, dst in ((wq, bqT, qT), (wk, bkT, kT)):
                    ps = psM.tile([128, 512], F32, tag="m")
                    for ct in range(NCT):
                        nc.tensor.matmul(ps[:], w_sb[:, ct], xT[:, ct, b, sl],
                                         start=(ct == 0), stop=(ct == NCT - 1))
                    nc.vector.tensor_scalar_add(dst[:, b, sl], ps[:], bias[:])
                # v^T, then transpose 128x128 blocks into s-major vhat
                ps = psM.tile([128, 512], F32, tag="m")
                for ct in range(NCT):
                    nc.tensor.matmul(ps[:], wv[:, ct], xT[:, ct, b, sl],
                                     start=(ct == 0), stop=(ct == NCT - 1))
                vt = vtp.tile([128, 512], DT_P, tag="vt")
                nc.vector.tensor_scalar_add(vt[:], ps[:], bvT[:])
                for sub in range(4):
                    st = qb * 4 + sub
                    tr = psM.tile([128, 128], DT_P, tag="m")
                    nc.tensor.transpose(
                        tr[:], vt[:, sub * 128:(sub + 1) * 128], ident[:])
                    nc.vector.tensor_copy(
                        vhat[:, b, st, :, 0:64],
                        tr[:].rearrange("p (hh d) -> p hh d", hh=HPC))

            def attn_chunk(hh, b, qb, dnm):
                hlo = hh * 64
                o_ps = psOC.tile([65, 512], F32, tag="o")
                nsi = 4 * qb + 4
                # chunks (si, lo): lo = in-block column offset; pack pairs
                # into one 2-bank PSUM tile so exp covers both
                chunks = [(si, 0) for si in range(4 * qb)] + \
                         [(si, si * 128 - qb * 512) for si in range(4 * qb, nsi)]
                groups = []
                i = 0
                while i < len(chunks):
                    w0 = 512 - chunks[i][1]
                    if i + 1 < len(chunks) and w0 + (512 - chunks[i + 1][1]) <= 1024:
                        groups.append([chunks[i], chunks[i + 1]])
                        i += 2
                    else:
                        groups.append([chunks[i]])
                        i += 1
                def emit_pv(grp, p_sb):
                    off = 0
                    for si, lo in grp:
                        w = 512 - lo
                        if lo > 0 or si * 128 == qb * 512:
                            # diagonal block: causal triangle mask on the
                            # otherwise-idle gpsimd engine (keeps the DVE
                            # queue out of the Exp->PV critical chain)
                            nc.gpsimd.tensor_tensor(
                                p_sb[:, off:off + 128], p_sb[:, off:off + 128],
                                tri[:], op=OP.mult)
                        nc.tensor.matmul(
                            o_ps[:, lo:512], vhat[:, b, si, hh, :],
                            p_sb[:, off:off + w],
                            start=(si == 0), stop=(si == nsi - 1))
                        off += w

                # one-group software pipeline: emit QK(g+1) before PV(g) so
                # the in-order PE queue never waits on Exp(g)
                pend = None
                for grp in groups:
                    tot = sum(512 - lo for _, lo in grp)
                    s_ps = psS2.tile([128, 1024], F32, tag="s2")
                    p_sb = pp.tile([128, 1024], DT_P, tag="p")
                    off = 0
                    for si, lo in grp:
                        w = 512 - lo
                        nc.tensor.matmul(
                            s_ps[:, off:off + w],
                            kT[hlo:hlo + 64, b, si * 128:(si + 1) * 128],
                            qT[hlo:hlo + 64, b, qb * 512 + lo:(qb + 1) * 512],
                            start=True, stop=True)
                        off += w
                    nc.scalar.activation(p_sb[:, 0:tot], s_ps[:, 0:tot],
                                         AF.Exp, scale=0.125)
                    if pend is not None:
                        emit_pv(*pend)
                    pend = (grp, p_sb)
                emit_pv(*pend)
                # cast the unnormalized chunk (65 rows incl denominator) to
                # SBUF bf16; gather the denominator row into dnm[qb] (DMA,
                # cross-partition) for the batched reciprocal
                oc = ohp.tile([65, 512], DT_A2A, tag="oh")
                nc.vector.tensor_copy(oc[:], o_ps[:])
                nc.sync.dma_start(dnm[qb:qb + 1, :], oc[64:65, :])
                return oc

            def attn_norm(hh, b, ocs, bcp):
                """softmax normalization at the source: one batched exact
                reciprocal on DVE (the HW reciprocal costs ~3.3us regardless
                of partition count, so batch all 4 q-blocks), broadcast
                across partitions via a K=1 matmul on the PE (outer product
                with a ones row), multiply in place on DVE, then ship."""
                dnm = ocs["dnm"]
                rcp = rp.tile([NQB, 512], F32, tag="rc")
                nc.vector.reciprocal(rcp[:], dnm[:])
                rcb = rp.tile([NQB, 512], DT_P, tag="rcb")
                nc.vector.tensor_copy(rcb[:], rcp[:])
                # PE operands must sit at base partition 0: flatten the four
                # reciprocal rows into one partition via DMA
                rcb1 = rp.tile([1, NQB, 512], DT_P, tag="rcb1")
                nc.sync.dma_start(rcb1[:], rcb[:])
                for qb in range(NQB):
                    oc = ocs[qb]
                    bc = bcp.tile([64, 512], F32, tag="m" if bcp is psM else "o")
                    nc.tensor.matmul(bc[:], ones1[0:1, 0:64],
                                     rcb1[0:1, qb, :], start=True, stop=True)
                    nc.vector.tensor_tensor(oc[0:64, :], oc[0:64, :], bc[:],
                                            op=OP.mult)
                    nc.sync.dma_start(a2a_in[hh][b * 4 + qb, :, :],
                                      oc[0:64, :])

            # ---- phase 1: proj + BOTH heads' attention interleaved per
            # (b, q-block); the scalar engine's Exp stream is the co-critical
            # resource, so head-1 work fills the slack under the projections
            # instead of sitting exposed after them.  Norm chains for batch b
            # are emitted after the first chunk of b+1 so their PE broadcasts
            # never block the in-order PE queue.  Both AllToAlls trigger
            # back-to-back at the end. ----
            prev = None
            for b in range(B):
                cur = []
                for hh in range(HPC):
                    dnm = rp.tile([NQB, 512], DT_A2A, tag="dn")
                    cur.append({"dnm": dnm})
                for qb in range(NQB):
                    proj(b, qb)
                    for hh in range(HPC):
                        cur[hh][qb] = attn_chunk(hh, b, qb, cur[hh]["dnm"])
                    if qb == 0 and prev is not None:
                        attn_norm(0, b - 1, prev[0], psOC)
                        attn_norm(1, b - 1, prev[1], psOC)
                prev = cur
            attn_norm(0, B - 1, prev[0], psM)
            nc.gpsimd.collective_compute(
                "AllToAll", OP.bypass, replica_groups=[list(range(NCORES))],
                ins=[a2a_in[0].opt()], outs=[a2a_out[0].opt()])
            attn_norm(1, B - 1, prev[1], psM)
            nc.gpsimd.collective_compute(
                "AllToAll", OP.bypass, replica_groups=[list(range(NCORES))],
                ins=[a2a_in[1].opt()], outs=[a2a_out[1].opt()])

            # ---- everything below depends on the collectives.  Pin it late
            # in the Tile scheduler's simulated timeline so none of it gets
            # slotted ahead of attention work on the in-order engine queues
            # (the scheduler's collective cost model is far too optimistic;
            # on HW an early-slotted dependent op stalls its whole queue). ----
            with tc.tile_wait_until(1.0):
                # o^T assembly: one strided DMA per head half, DRAM->SBUF
                for j in range(2):
                    nc.sync.dma_start(
                        oT[j * 64:(j + 1) * 64, :, :],
                        a2a_out[j][:, :, :].rearrange("f p c -> p f c"))

                # preload the Sqrt activation table while waiting for the
                # second AllToAll (keeps the load off tt0's critical chain)
                sqw = lnp.tile([1, 1], F32, tag="sqw")
                nc.scalar.activation(sqw[:], oT[0:1, 0, 0:1], AF.Sqrt)

                # PE p-state warmup: a bounded accumulation chain reading
                # the just-assembled head-0 half of oT.  It runs in the gap
                # between the two AllToAlls and brings the PE clock back up
                # before the real output projection starts.
                wup = psM.tile([128, 512], F32, tag="m")
                for i in range(30):
                    ft = i % NCT
                    nc.tensor.matmul(wup[:], oT[0:64, ft, 0:128],
                                     wo[0:64, ft, 0:512],
                                     start=(i == 0), stop=(i == 29))

                # ---- phase 4: out-proj + bias + LayerNorm ----
                for tt in range(TS // 128):
                    y2 = psS2.tile([128, 1024], F32, tag="s2")
                    yc = lnp.tile([128, C], BF16, tag="yc")
                    s0 = lnp.tile([128, 1], F32, tag="s0")
                    s1 = lnp.tile([128, 1], F32, tag="s1")
                    q0 = lnp.tile([128, 1], F32, tag="q0")
                    q1 = lnp.tile([128, 1], F32, tag="q1")
                    for nb, (s_acc, q_acc) in enumerate(((s0, q0), (s1, q1))):
                        half = slice(nb * 512, (nb + 1) * 512)
                        yh = y2[:, half]
                        for ft in range(NCT):
                            nc.tensor.matmul(
                                yh, oT[:, ft, tt * 128:(tt + 1) * 128],
                                wo[:, ft, nb * 512:(nb + 1) * 512],
                                start=(ft == 0), stop=False)
                        nc.tensor.matmul(yh, ones1[:], bo[:, half],
                                         start=False, stop=True)
                        # move to SBUF + row-sum on the scalar engine (Copy
                        # is in every ACT table set -- no table switching)
                        nc.scalar.activation(yc[:, half], yh, AF.Copy,
                                             accum_out=s_acc[:])
                        # sum of squares on the scalar engine
                        sqh = lnp.tile([128, 512], BF16, tag=f"sqh{nb}")
                        nc.scalar.activation(sqh[:], yh, AF.Square,
                                             accum_out=q_acc[:])
                    # fused LN stats: mu=(s0+s1)/C, qe=(q0+q1)+C*eps,
                    # m2n=-mu^2, sd=sqrt(qe/C + m2n) == sqrt(var+eps)
                    mu = lnp.tile([128, 1], F32, tag="mu")
                    nc.vector.tensor_scalar(mu[:], s0[:], s1[:], 1.0 / C,
                                            op0=OP.add, op1=OP.mult)
                    qe = lnp.tile([128, 1], F32, tag="qe")
                    nc.vector.tensor_scalar(qe[:], q0[:], q1[:], C * EPS,
                                            op0=OP.add, op1=OP.add)
                    m2n = lnp.tile([128, 1], F32, tag="m2n")
                    nc.vector.tensor_scalar(m2n[:], mu[:], mu[:], -1.0,
                                            op0=OP.mult, op1=OP.mult)
                    sd = lnp.tile([128, 1], F32, tag="sd")
                    nc.scalar.activation(sd[:], qe[:], AF.Sqrt,
                                         scale=1.0 / C, bias=m2n[:])
                    istd = lnp.tile([128, 1], F32, tag="istd")
                    nc.vector.reciprocal(istd[:], sd[:])
                    yn = lnp.tile([128, C], BF16, tag="yn")
                    nc.vector.tensor_scalar(
                        yn[:], yc[:], mu[:], istd[:], op0=OP.subtract, op1=OP.mult)
                    yg = lnp.tile([128, C], BF16, tag="yg")
                    nc.vector.tensor_tensor(yg[:], yn[:], gam[:], op=OP.mult)
                    yf = lnp.tile([128, C], BF16, tag="yf")
                    nc.vector.tensor_tensor(yf[:], yg[:], bet[:], op=OP.add)
                    nc.sync.dma_start(out_h[tt * 128:(tt + 1) * 128, :], yf[:])

    nc.compile()
    return nc


def _get_nc():
    if "nc" not in _CACHE:
        _CACHE["nc"] = _build()
    return _CACHE["nc"]


def _tile_w(w):
    m = w.shape[1]
    return np.ascontiguousarray(
        w.reshape(NCT, 128, m).transpose(1, 0, 2)).astype(NP_W)


def _make_in_maps(inputs):
    x = np.asarray(inputs["x"], np.float32)
    Wq = np.asarray(inputs["Wq"], np.float32)
    Wk = np.asarray(inputs["Wk"], np.float32)
    Wv = np.asarray(inputs["Wv"], np.float32)
    Wo = np.asarray(inputs["Wo"], np.float32)
    bq = np.asarray(inputs["bq"], np.float32)
    bk = np.asarray(inputs["bk"], np.float32)
    bv = np.asarray(inputs["bv"], np.float32)
    bo = np.asarray(inputs["bo"], np.float32)
    gamma = np.asarray(inputs["gamma"], np.float32)
    beta = np.asarray(inputs["beta"], np.float32)

    # [C, B, T] pre-tiled to [128, NCT, B, T] (partition-major)
    xT = np.ascontiguousarray(
        x.transpose(2, 0, 1).reshape(NCT, 128, B, T).transpose(1, 0, 2, 3)
    ).astype(NP_X)
    wo_c = np.ascontiguousarray(
        Wo.reshape(NCT, 128, C).transpose(1, 0, 2)).astype(NP_W)
    bo_row = np.ascontiguousarray(bo.reshape(1, C)).astype(ml_dtypes.bfloat16)
    gamb = np.ascontiguousarray(np.broadcast_to(gamma, (128, C))).astype(ml_dtypes.bfloat16)
    betb = np.ascontiguousarray(np.broadcast_to(beta, (128, C))).astype(ml_dtypes.bfloat16)

    maps = []
    for i in range(NCORES):
        cols = slice(DPC * i, DPC * (i + 1))
        maps.append({
            "xT": xT,
            "wq": _tile_w(Wq[:, cols]),
            "wk": _tile_w(Wk[:, cols]),
            "wv": _tile_w(Wv[:, cols]),
            "wo": wo_c,
            "bqT": np.ascontiguousarray(bq[cols].reshape(DPC, 1)),
            "bkT": np.ascontiguousarray(bk[cols].reshape(DPC, 1)),
            "bvT": np.ascontiguousarray(bv[cols].reshape(DPC, 1)),
            "bo_row": bo_row,
            "gamb": gamb,
            "betb": betb,
        })
    return maps


def _run(inputs, trace=False, **kwargs):
    nc = _get_nc()
    in_maps = _make_in_maps(inputs)
    res = run_bass_kernel_spmd(nc, in_maps, core_ids=list(range(NCORES)),
                               trace=trace, **kwargs)
    y = np.empty((B, T, C), np.float32)
    for i in range(NCORES):
        b, ts = divmod(i, 4)
        y[b, ts * TS:(ts + 1) * TS, :] = res.results[i]["out"].astype(np.float32)
    return y, res


def kernel(**inputs) -> np.ndarray:
    out, _ = _run(inputs, trace=False)
    return out


# revision 30
# speedup vs baseline: 1.0274x; 1.0274x over previous
"""Distributed Trainium2 Bass kernel for a causal attention block + LayerNorm.

Reference computation (B=2, T=2048, C=1024, H=16 heads, Dh=64):
    q,k,v = x@Wq+bq, x@Wk+bk, x@Wv+bv          (per-head split)
    att   = softmax(causal(q k^T / sqrt(Dh)))
    o     = att @ v ; y = o@Wo + bo ; out = LayerNorm(y) * gamma + beta

Sharding (8 cores, one TRN2 chip):
    Tensor-parallel over heads: core i owns heads {2i, 2i+1} for BOTH
    batches (Megatron-style column shards of Wq/Wk/Wv).  After attention,
    two 8-core AllToAlls (one per local head, bf16 payload) redistribute the
    per-head outputs to token-sharding: core i ends with tokens
    [b = i//4, t in (i%4)*512 ...] with ALL 1024 features, applies the
    output projection (full Wo), bias and LayerNorm locally, and writes its
    (512, 1024) slice of the output.

Key scheduling facts this version is built around (measured on HW):
    - cores launch with ~80us of skew; every collective rendezvous waits for
      the straggler.  The wall time is ~(skew + per-core serial path), so the
      kernel minimizes the per-core path and keeps all collective-dependent
      work strictly AFTER independent work on every engine queue.
    - the Tile scheduler trusts its (fast) collective cost model and will
      happily slot post-collective ops between attention ops on the in-order
      engine queues; on HW they block on the real collective and stall the
      rest of attention.  tc.tile_wait_until() pushes the tail out.
    - softmax normalization is applied at the SOURCE core: denominators for
      all four q-blocks of a (head, batch) are gathered into one tile and
      inverted with a single exact DVE reciprocal (the HW op costs ~3.3us
      regardless of partition count), broadcast across partitions with a
      K=1 outer-product matmul on the PE, and multiplied in on DVE -- so
      after the AllToAlls only DMAs + the output projection remain.

Layout choices (all on-chip matmuls contract over the partition axis):
    - activations are feature-major: host passes x^T [C, B, T].
    - q^T,k^T,v^T [d, t] produced directly; v transposed on the PE into
      s-major v-hat [s, d] with an extra ones column per head so the P@V
      matmul also yields the softmax denominator for free.
    - scores are computed transposed: S^T[s, q] = k^T.T @ q^T; score chunks
      are packed in pairs into 2-bank PSUM tiles so each scalar-engine Exp
      call covers up to 1024 columns; causal masking via a triangular
      bf16 multiply on the 128-wide diagonal blocks only (on DVE).
"""

import numpy as np
import ml_dtypes

import concourse.bass as bass
import concourse.mybir as mybir
import concourse.tile as tile
from concourse import bacc
from concourse.bass_utils import run_bass_kernel_spmd
F32 = mybir.dt.float32
BF16 = mybir.dt.bfloat16
AF = mybir.ActivationFunctionType
OP = mybir.AluOpType

B, T, C, H, Dh = 2, 2048, 1024, 16, 64
NCORES = 8
HPC = 2               # heads per core
DPC = HPC * Dh        # 128 feature columns per core
TS = 512              # output token-slice length per core
NQB = T // 512        # 4 q blocks
NST = T // 128        # 16 s tiles
NCT = C // 128        # 8 contraction tiles
EPS = 1e-5

DT_X = BF16
DT_W = BF16
DT_P = BF16
DT_A2A = BF16         # AllToAll payload dtype
NP_X = ml_dtypes.bfloat16
NP_W = ml_dtypes.bfloat16

_CACHE = {}


def _build():
    nc = bacc.Bacc("TRN2", target_bir_lowering=False, debug=False,
                   num_devices=NCORES)

    xT_h = nc.dram_tensor("xT", [128, NCT, B, T], DT_X, kind="ExternalInput")
    wq_h = nc.dram_tensor("wq", [128, NCT, DPC], DT_W, kind="ExternalInput")
    wk_h = nc.dram_tensor("wk", [128, NCT, DPC], DT_W, kind="ExternalInput")
    wv_h = nc.dram_tensor("wv", [128, NCT, DPC], DT_W, kind="ExternalInput")
    wo_h = nc.dram_tensor("wo", [128, NCT, C], DT_W, kind="ExternalInput")
    bqT_h = nc.dram_tensor("bqT", [DPC, 1], F32, kind="ExternalInput")
    bkT_h = nc.dram_tensor("bkT", [DPC, 1], F32, kind="ExternalInput")
    bvT_h = nc.dram_tensor("bvT", [DPC, 1], F32, kind="ExternalInput")
    bo_h = nc.dram_tensor("bo_row", [1, C], BF16, kind="ExternalInput")
    gam_h = nc.dram_tensor("gamb", [128, C], BF16, kind="ExternalInput")
    bet_h = nc.dram_tensor("betb", [128, C], BF16, kind="ExternalInput")
    out_h = nc.dram_tensor("out", [TS, C], BF16, kind="ExternalOutput")

    ones1_d = nc.inline_tensor(np.ones((1, 128), ml_dtypes.bfloat16), name="ones1_const")
    ident_d = nc.inline_tensor(
        np.eye(128, dtype=ml_dtypes.bfloat16), name="ident_const")
    tri_np = (np.tril(np.ones((128, 128), np.float32)).T).astype(ml_dtypes.bfloat16)
    tri_d = nc.inline_tensor(tri_np, name="tri_const")

    with tile.TileContext(nc) as tc:
        with (
            tc.tile_pool(name="const", bufs=1) as cp,
            tc.tile_pool(name="dram", bufs=1, space="DRAM") as dp,
            tc.tile_pool(name="act", bufs=1) as ap,
            tc.tile_pool(name="wop", bufs=1) as wop,
            tc.tile_pool(name="pp", bufs=5) as pp,
            tc.tile_pool(name="vtp", bufs=2) as vtp,
            tc.tile_pool(name="ohp", bufs=18) as ohp,
            tc.tile_pool(name="rp", bufs=4) as rp,
            tc.tile_pool(name="xw", bufs=1) as xw,
            tc.tile_pool(name="lnp", bufs=2) as lnp,
            tc.tile_pool(name="psM", bufs=2, space="PSUM") as psM,
            tc.tile_pool(name="psS2", bufs=2, space="PSUM") as psS2,
            tc.tile_pool(name="psOC", bufs=2, space="PSUM") as psOC,
        ):
            # ---- weights + first x chunk on the sync queue (needed first),
            # rest of x^T + output-side weights on the gpsimd queue ----
            wq = xw.tile([128, NCT, DPC], DT_W)
            wk = xw.tile([128, NCT, DPC], DT_W)
            wv = xw.tile([128, NCT, DPC], DT_W)
            xT = xw.tile([128, NCT, B, T], DT_X)
            nc.gpsimd.dma_start(wv[:], wv_h[:])
            bqT = cp.tile([DPC, 1], F32)
            nc.scalar.dma_start(bqT[:], bqT_h[:])
            bkT = cp.tile([DPC, 1], F32)
            nc.scalar.dma_start(bkT[:], bkT_h[:])
            bvT = cp.tile([DPC, 1], F32)
            nc.scalar.dma_start(bvT[:], bvT_h[:])
            ident = cp.tile([128, 128], BF16)
            nc.scalar.dma_start(ident[:], ident_d[:])
            ones1 = cp.tile([1, 128], BF16)
            nc.scalar.dma_start(ones1[:], ones1_d[:])
            tri = cp.tile([128, 128], BF16)
            nc.scalar.dma_start(tri[:], tri_d[:])
            nc.sync.dma_start(wq[:], wq_h[:])
            nc.sync.dma_start(xT[:, 0, 0, 0:512], xT_h[:, 0, 0, 0:512])
            nc.sync.dma_start(wk[:], wk_h[:])
            for ct in range(1, NCT):
                nc.sync.dma_start(xT[:, ct, 0, 0:512], xT_h[:, ct, 0, 0:512])

            # tiny warm-up AllToAll: trigger it before the bulk DMA issues
            # clog the gpsimd queue, so the first-collective overhead is paid
            # as early as possible
            warm_in = dp.tile([NCORES, 8], F32, tag="wi")
            warm_out = dp.tile([NCORES, 8], F32, tag="wo_")
            warm_sb = cp.tile([NCORES, 8], F32)
            nc.gpsimd.memset(warm_sb[:], 0.0)
            nc.gpsimd.dma_start(warm_in[:], warm_sb[:])
            nc.gpsimd.collective_compute(
                "AllToAll", OP.bypass, replica_groups=[list(range(NCORES))],
                ins=[warm_in.opt()], outs=[warm_out.opt()])

            # split the b0-remainder stream across two DMA queues so the
            # projection loop never outruns the x data
            for ct in range(NCT):
                q = nc.gpsimd if ct % 2 == 0 else nc.scalar
                q.dma_start(xT[:, ct, 0, 512:T], xT_h[:, ct, 0, 512:T])
            for ct in range(NCT):
                nc.gpsimd.dma_start(xT[:, ct, 1, :], xT_h[:, ct, 1, :])

            # output-side weights, early (small; they only cost SBUF)
            wo = wop.tile([128, NCT, C], DT_W, tag="wo")
            nc.gpsimd.dma_start(wo[:], wo_h[:])
            bo = wop.tile([1, C], BF16, tag="bo")
            nc.gpsimd.dma_start(bo[:], bo_h[:])
            gam = wop.tile([128, C], BF16, tag="gam")
            nc.gpsimd.dma_start(gam[:], gam_h[:])
            bet = wop.tile([128, C], BF16, tag="bet")
            nc.gpsimd.dma_start(bet[:], bet_h[:])
            eps_t = wop.tile([128, 1], F32, tag="eps")
            nc.gpsimd.memset(eps_t[:], EPS)

            # ---- persistent activation tiles ----
            qT = ap.tile([DPC, B, T], DT_P)
            kT = ap.tile([DPC, B, T], DT_P)
            vhat = ap.tile([128, B, NST, HPC, 65], DT_P)
            oT = ap.tile([128, NCT, 512], DT_P)
            for b in range(B):
                nc.gpsimd.memset(vhat[:, b, :, :, 64:65], 1.0)

            a2a_in0 = dp.tile([NCORES, 64, 512], DT_A2A, tag="ai0")
            a2a_in1 = dp.tile([NCORES, 64, 512], DT_A2A, tag="ai1")
            a2a_out0 = dp.tile([NCORES, 64, 512], DT_A2A, tag="ao0")
            a2a_out1 = dp.tile([NCORES, 64, 512], DT_A2A, tag="ao1")
            a2a_in = [a2a_in0, a2a_in1]
            a2a_out = [a2a_out0, a2a_out1]

            def proj(b, qb):
                sl = slice(qb * 512, (qb + 1) * 512)
                for w_sb, bias, dst in ((wq, bqT, qT), (wk, bkT, kT)):
                    ps = psM.tile([128, 512], F32, tag="m")
                    for ct in range(NCT):
                        nc.tensor.matmul(ps[:], w_sb[:, ct], xT[:, ct, b, sl],
                                         start=(ct == 0), stop=(ct == NCT - 1))
                    nc.vector.tensor_scalar_add(dst[:, b, sl], ps[:], bias[:])
                # v^T, then transpose 128x128 blocks into s-major vhat
                ps = psM.tile([128, 512], F32, tag="m")
                for ct in range(NCT):
                    nc.tensor.matmul(ps[:], wv[:, ct], xT[:, ct, b, sl],
                                     start=(ct == 0), stop=(ct == NCT - 1))
                vt = vtp.tile([128, 512], DT_P, tag="vt")
                nc.vector.tensor_scalar_add(vt[:], ps[:], bvT[:])
                for sub in range(4):
                    st = qb * 4 + sub
                    tr = psM.tile([128, 128], DT_P, tag="m")
                    nc.tensor.transpose(
                        tr[:], vt[:, sub * 128:(sub + 1) * 128], ident[:])
                    nc.vector.tensor_copy(
                        vhat[:, b, st, :, 0:64],
                        tr[:].rearrange("p (hh d) -> p hh d", hh=HPC))

            def attn_chunk(hh, b, qb, dnm):
                hlo = hh * 64
                o_ps = psOC.tile([65, 512], F32, tag="o")
                nsi = 4 * qb + 4
                # chunks (si, lo): lo = in-block column offset; pack pairs
                # into one 2-bank PSUM tile so exp covers both
                chunks = [(si, 0) for si in range(4 * qb)] + \
                         [(si, si * 128 - qb * 512) for si in range(4 * qb, nsi)]
                groups = []
                i = 0
                while i < len(chunks):
                    w0 = 512 - chunks[i][1]
                    if i + 1 < len(chunks) and w0 + (512 - chunks[i + 1][1]) <= 1024:
                        groups.append([chunks[i], chunks[i + 1]])
                        i += 2
                    else:
                        groups.append([chunks[i]])
                        i += 1
                def emit_pv(grp, p_sb):
                    off = 0
                    for si, lo in grp:
                        w = 512 - lo
                        if lo > 0 or si * 128 == qb * 512:
                            # diagonal block: causal triangle mask on the
                            # otherwise-idle gpsimd engine (keeps the DVE
                            # queue out of the Exp->PV critical chain)
                            nc.gpsimd.tensor_tensor(
                                p_sb[:, off:off + 128], p_sb[:, off:off + 128],
                                tri[:], op=OP.mult)
                        nc.tensor.matmul(
                            o_ps[:, lo:512], vhat[:, b, si, hh, :],
                            p_sb[:, off:off + w],
                            start=(si == 0), stop=(si == nsi - 1))
                        off += w

                # one-group software pipeline: emit QK(g+1) before PV(g) so
                # the in-order PE queue never waits on Exp(g)
                pend = None
                for grp in groups:
                    tot = sum(512 - lo for _, lo in grp)
                    s_ps = psS2.tile([128, 1024], F32, tag="s2")
                    p_sb = pp.tile([128, 1024], DT_P, tag="p")
                    off = 0
                    for si, lo in grp:
                        w = 512 - lo
                        nc.tensor.matmul(
                            s_ps[:, off:off + w],
                            kT[hlo:hlo + 64, b, si * 128:(si + 1) * 128],
                            qT[hlo:hlo + 64, b, qb * 512 + lo:(qb + 1) * 512],
                            start=True, stop=True)
                        off += w
                    nc.scalar.activation(p_sb[:, 0:tot], s_ps[:, 0:tot],
                                         AF.Exp, scale=0.125)
                    if pend is not None:
                        emit_pv(*pend)
                    pend = (grp, p_sb)
                emit_pv(*pend)
                # cast the unnormalized chunk (65 rows incl denominator) to
                # SBUF bf16; gather the denominator row into dnm[qb] (DMA,
                # cross-partition) for the batched reciprocal
                oc = ohp.tile([65, 512], DT_A2A, tag="oh")
                nc.vector.tensor_copy(oc[:], o_ps[:])
                nc.sync.dma_start(dnm[qb:qb + 1, :], oc[64:65, :])
                return oc

            def attn_norm(hh, b, ocs, bcp):
                """softmax normalization at the source: one batched exact
                reciprocal on DVE (the HW reciprocal costs ~3.3us regardless
                of partition count, so batch all 4 q-blocks), broadcast
                across partitions via a K=1 matmul on the PE (outer product
                with a ones row), multiply in place on DVE, then ship."""
                dnm = ocs["dnm"]
                rcp = rp.tile([NQB, 512], F32, tag="rc")
                nc.vector.reciprocal(rcp[:], dnm[:])
                rcb = rp.tile([NQB, 512], DT_P, tag="rcb")
                nc.vector.tensor_copy(rcb[:], rcp[:])
                # PE operands must sit at base partition 0: flatten the four
                # reciprocal rows into one partition via DMA
                rcb1 = rp.tile([1, NQB, 512], DT_P, tag="rcb1")
                nc.sync.dma_start(rcb1[:], rcb[:])
                for qb in range(NQB):
                    oc = ocs[qb]
                    bc = bcp.tile([64, 512], F32, tag="m" if bcp is psM else "o")
                    nc.tensor.matmul(bc[:], ones1[0:1, 0:64],
                                     rcb1[0:1, qb, :], start=True, stop=True)
                    nc.vector.tensor_tensor(oc[0:64, :], oc[0:64, :], bc[:],
                                            op=OP.mult)
                    nc.sync.dma_start(a2a_in[hh][b * 4 + qb, :, :],
                                      oc[0:64, :])

            # ---- phase 1: proj + BOTH heads' attention interleaved per
            # (b, q-block); the scalar engine's Exp stream is the co-critical
            # resource, so head-1 work fills the slack under the projections
            # instead of sitting exposed after them.  Norm chains for batch b
            # are emitted after the first chunk of b+1 so their PE broadcasts
            # never block the in-order PE queue.  Both AllToAlls trigger
            # back-to-back at the end. ----
            prev = None
            for b in range(B):
                cur = []
                for hh in range(HPC):
                    dnm = rp.tile([NQB, 512], DT_A2A, tag="dn")
                    cur.append({"dnm": dnm})
                for qb in range(NQB):
                    proj(b, qb)
                    for hh in range(HPC):
                        cur[hh][qb] = attn_chunk(hh, b, qb, cur[hh]["dnm"])
                    if qb == 0 and prev is not None:
                        attn_norm(0, b - 1, prev[0], psOC)
                        attn_norm(1, b - 1, prev[1], psOC)
                prev = cur
            attn_norm(0, B - 1, prev[0], psM)
            nc.gpsimd.collective_compute(
                "AllToAll", OP.bypass, replica_groups=[list(range(NCORES))],
                ins=[a2a_in[0].opt()], outs=[a2a_out[0].opt()])
            attn_norm(1, B - 1, prev[1], psM)
            nc.gpsimd.collective_compute(
                "AllToAll", OP.bypass, replica_groups=[list(range(NCORES))],
                ins=[a2a_in[1].opt()], outs=[a2a_out[1].opt()])

            # ---- everything below depends on the collectives.  Pin it late
            # in the Tile scheduler's simulated timeline so none of it gets
            # slotted ahead of attention work on the in-order engine queues
            # (the scheduler's collective cost model is far too optimistic;
            # on HW an early-slotted dependent op stalls its whole queue). ----
            with tc.tile_wait_until(1.0):
                # o^T assembly: one strided DMA per head half, DRAM->SBUF
                for j in range(2):
                    nc.sync.dma_start(
                        oT[j * 64:(j + 1) * 64, :, :],
                        a2a_out[j][:, :, :].rearrange("f p c -> p f c"))

                # preload the Sqrt activation table while waiting for the
                # second AllToAll (keeps the load off tt0's critical chain)
                sqw = lnp.tile([1, 1], F32, tag="sqw")
                nc.scalar.activation(sqw[:], oT[0:1, 0, 0:1], AF.Sqrt)

                # PE p-state warmup: a bounded accumulation chain reading
                # the just-assembled head-0 half of oT.  It runs in the gap
                # between the two AllToAlls and brings the PE clock back up
                # before the real output projection starts.
                wup = psM.tile([128, 512], F32, tag="m")
                for i in range(30):
                    ft = i % NCT
                    nc.tensor.matmul(wup[:], oT[0:64, ft, 0:128],
                                     wo[0:64, ft, 0:512],
                                     start=(i == 0), stop=(i == 29))

                # ---- phase 4: out-proj + bias + LayerNorm ----
                for tt in range(TS // 128):
                    y2 = psS2.tile([128, 1024], F32, tag="s2")
                    yc = lnp.tile([128, C], BF16, tag="yc")
                    s0 = lnp.tile([128, 1], F32, tag="s0")
                    s1 = lnp.tile([128, 1], F32, tag="s1")
                    q0 = lnp.tile([128, 1], F32, tag="q0")
                    q1 = lnp.tile([128, 1], F32, tag="q1")
                    for nb, (s_acc, q_acc) in enumerate(((s0, q0), (s1, q1))):
                        half = slice(nb * 512, (nb + 1) * 512)
                        yh = y2[:, half]
                        for ft in range(NCT):
                            nc.tensor.matmul(
                                yh, oT[:, ft, tt * 128:(tt + 1) * 128],
                                wo[:, ft, nb * 512:(nb + 1) * 512],
                                start=(ft == 0), stop=False)
                        nc.tensor.matmul(yh, ones1[:], bo[:, half],
                                         start=False, stop=True)
                        # move to SBUF + row-sum on the scalar engine (Copy
                        # is in every ACT table set -- no table switching)
                        nc.scalar.activation(yc[:, half], yh, AF.Copy,
                                             accum_out=s_acc[:])
                        # sum of squares on the scalar engine
                        sqh = lnp.tile([128, 512], BF16, tag=f"sqh{nb}")
                        nc.scalar.activation(sqh[:], yh, AF.Square,
                                             accum_out=q_acc[:])
                    # fused LN stats: mu=(s0+s1)/C, qe=(q0+q1)+C*eps,
                    # m2n=-mu^2, sd=sqrt(qe/C + m2n) == sqrt(var+eps)
                    mu = lnp.tile([128, 1], F32, tag="mu")
                    nc.vector.tensor_scalar(mu[:], s0[:], s1[:], 1.0 / C,
                                            op0=OP.add, op1=OP.mult)
                    qe = lnp.tile([128, 1], F32, tag="qe")
                    nc.vector.tensor_scalar(qe[:], q0[:], q1[:], C * EPS,
                                            op0=OP.add, op1=OP.add)
                    m2n = lnp.tile([128, 1], F32, tag="m2n")
                    nc.vector.tensor_scalar(m2n[:], mu[:], mu[:], -1.0,
                                            op0=OP.mult, op1=OP.mult)
                    sd = lnp.tile([128, 1], F32, tag="sd")
                    nc.scalar.activation(sd[:], qe[:], AF.Sqrt,
                                         scale=1.0 / C, bias=m2n[:])
                    istd = lnp.tile([128, 1], F32, tag="istd")
                    nc.vector.reciprocal(istd[:], sd[:])
                    yn = lnp.tile([128, C], BF16, tag="yn")
                    nc.vector.tensor_scalar(
                        yn[:], yc[:], mu[:], istd[:], op0=OP.subtract, op1=OP.mult)
                    yg = lnp.tile([128, C], BF16, tag="yg")
                    nc.vector.tensor_tensor(yg[:], yn[:], gam[:], op=OP.mult)
                    yf = lnp.tile([128, C], BF16, tag="yf")
                    nc.vector.tensor_tensor(yf[:], yg[:], bet[:], op=OP.add)
                    nc.sync.dma_start(out_h[tt * 128:(tt + 1) * 128, :], yf[:])

    nc.compile()
    return nc


def _get_nc():
    if "nc" not in _CACHE:
        _CACHE["nc"] = _build()
    return _CACHE["nc"]


def _tile_w(w):
    m = w.shape[1]
    return np.ascontiguousarray(
        w.reshape(NCT, 128, m).transpose(1, 0, 2)).astype(NP_W)


def _make_in_maps(inputs):
    x = np.asarray(inputs["x"], np.float32)
    Wq = np.asarray(inputs["Wq"], np.float32)
    Wk = np.asarray(inputs["Wk"], np.float32)
    Wv = np.asarray(inputs["Wv"], np.float32)
    Wo = np.asarray(inputs["Wo"], np.float32)
    bq = np.asarray(inputs["bq"], np.float32)
    bk = np.asarray(inputs["bk"], np.float32)
    bv = np.asarray(inputs["bv"], np.float32)
    bo = np.asarray(inputs["bo"], np.float32)
    gamma = np.asarray(inputs["gamma"], np.float32)
    beta = np.asarray(inputs["beta"], np.float32)

    # [C, B, T] pre-tiled to [128, NCT, B, T] (partition-major)
    xT = np.ascontiguousarray(
        x.transpose(2, 0, 1).reshape(NCT, 128, B, T).transpose(1, 0, 2, 3)
    ).astype(NP_X)
    wo_c = np.ascontiguousarray(
        Wo.reshape(NCT, 128, C).transpose(1, 0, 2)).astype(NP_W)
    bo_row = np.ascontiguousarray(bo.reshape(1, C)).astype(ml_dtypes.bfloat16)
    gamb = np.ascontiguousarray(np.broadcast_to(gamma, (128, C))).astype(ml_dtypes.bfloat16)
    betb = np.ascontiguousarray(np.broadcast_to(beta, (128, C))).astype(ml_dtypes.bfloat16)

    maps = []
    for i in range(NCORES):
        cols = slice(DPC * i, DPC * (i + 1))
        maps.append({
            "xT": xT,
            "wq": _tile_w(Wq[:, cols]),
            "wk": _tile_w(Wk[:, cols]),
            "wv": _tile_w(Wv[:, cols]),
            "wo": wo_c,
            "bqT": np.ascontiguousarray(bq[cols].reshape(DPC, 1)),
            "bkT": np.ascontiguousarray(bk[cols].reshape(DPC, 1)),
            "bvT": np.ascontiguousarray(bv[cols].reshape(DPC, 1)),
            "bo_row": bo_row,
            "gamb": gamb,
            "betb": betb,
        })
    return maps


def _run(inputs, trace=False, **kwargs):
    nc = _get_nc()
    in_maps = _make_in_maps(inputs)
    res = run_bass_kernel_spmd(nc, in_maps, core_ids=list(range(NCORES)),
                               trace=trace, **kwargs)
    y = np.empty((B, T, C), np.float32)
    for i in range(NCORES):
        b, ts = divmod(i, 4)
        y[b, ts * TS:(ts + 1) * TS, :] = res.results[i]["out"].astype(np.float32)
    return y, res


def kernel(**inputs) -> np.ndarray:
    out, _ = _run(inputs, trace=False)
    return out
